# revision 41
# baseline (speedup 1.0000x reference)
"""Trainium2 Bass kernel for nn_Attention_43946105373274.

Causal multi-head attention with rotary embeddings applied to q, k and v.
B=2, N=2048, DIM=1024, H=16, DH=64, f32.

Sharding: 8 cores = (2 batches) x (4 head-groups of 4 heads).
Each core computes the qkv projection for its heads (w_qkv column-shard),
full causal attention for its heads, and a partial output projection
(w_out row-shard).  The host sums the 4 partials per batch and adds the
bias -- full inputs in, full output out.

v2 design notes (vs the 240us baseline):
  - All matmuls run in the 128x128 PE tile mode: per-head k tiles are
    zero-padded to 128 contraction rows (kTz) so the S^T matmuls no
    longer flip the array into 64x128 mode (mode switches drain the PE).
  - No PE transposes at all: cos/sin for the [d, n] layout are computed
    from a host-folded freqs^T input and duplicated across head-halves
    with SBUF->SBUF DMAs; V is computed directly in [n, d] layout with
    w_v as the moving operand and an even/odd-permuted column order so
    rotate_half becomes contiguous half-tile DVE ops.
  - Bias matmuls removed (host adds b_out during the partial-sum gather).
  - Software pipelining: pair-1 QKV runs as PE filler inside pair-0's
    scalar-bound attention; the output projection overlaps pair-1's
    attention.  x^T DMA transposes split across the sync+scalar HWDGE
    queues.
  - Output written as bf16 (partials summed in f32 on the host).
"""

import sys
import numpy as np

if "/opt/trn_rl_repo" not in sys.path:
    sys.path.insert(0, "/opt/trn_rl_repo")

B, N, DIM, H, DH = 2, 2048, 1024, 16, 64
HPC = 4                     # heads per core
NCORES = 8
SCALE = DH ** -0.5
NT = N // 128               # 16 row tiles
KB = DIM // 128             # 8 contraction blocks
CW = 512                    # i-chunk width
NCH = N // CW               # 4 chunks

_CACHE = {}


def _build_program():
    import concourse.bass as bass  # noqa: F401
    import concourse.mybir as mybir
    import concourse.tile as tile
    from concourse import bacc

    F32 = mybir.dt.float32
    F32R = mybir.dt.float32r
    BF16 = mybir.dt.bfloat16
    AF = mybir.ActivationFunctionType
    OP = mybir.AluOpType

    nc = bacc.Bacc("TRN2", target_bir_lowering=False, debug=False,
                   num_devices=NCORES)

    xbT = nc.dram_tensor("xbT", [DIM, N], BF16, kind="ExternalInput")
    wqkD = nc.dram_tensor("wqk", [DIM, 4 * 128], BF16, kind="ExternalInput")
    wvD = nc.dram_tensor("wv", [DIM, 2 * 128], BF16, kind="ExternalInput")
    woD = nc.dram_tensor("wo", [HPC * DH, DIM], BF16, kind="ExternalInput")
    fTFD = nc.dram_tensor("fTF", [128, N // 2], F32, kind="ExternalInput")
    fPD = nc.dram_tensor("fP", [N, DH], F32, kind="ExternalInput")
    rmatD = nc.dram_tensor("rmatD", [128, 128], BF16, kind="ExternalInput")
    outD = nc.dram_tensor("out", [N, DIM], BF16, kind="ExternalOutput")
    import os
    dbg = os.environ.get("KDEBUG", "0") == "1"
    if dbg:
        dbgD = {
            "d_cosT2": nc.dram_tensor("d_cosT2", [128, N], F32, kind="ExternalOutput"),
            "d_sinT2": nc.dram_tensor("d_sinT2", [128, N], F32, kind="ExternalOutput"),
            "d_cosP": nc.dram_tensor("d_cosP", [128, NT * DH], F32, kind="ExternalOutput"),
            "d_qT0": nc.dram_tensor("d_qT0", [128, N], BF16, kind="ExternalOutput"),
            "d_kTz0": nc.dram_tensor("d_kTz0", [128, N], BF16, kind="ExternalOutput"),
            "d_kTz1": nc.dram_tensor("d_kTz1", [128, N], BF16, kind="ExternalOutput"),
            "d_vtall": nc.dram_tensor("d_vtall", [128, HPC * NT * (DH + 1)], BF16, kind="ExternalOutput"),
            "d_oT0": nc.dram_tensor("d_oT0", [128, N], BF16, kind="ExternalOutput"),
            "d_vraw": nc.dram_tensor("d_vraw", [128, 512], F32, kind="ExternalOutput"),
        }

    MAGIC = 12582912.0          # 1.5 * 2**23: float32 round-to-nearest trick
    TWO_PI = float(2 * np.pi)

    with tile.TileContext(nc) as tc:
        with tc.tile_pool(name="pc", bufs=1) as pc, \
             tc.tile_pool(name="pw", bufs=16) as pw, \
             tc.tile_pool(name="pwo", bufs=2) as pwo, \
             tc.tile_pool(name="pxT", bufs=8) as pxT, \
             tc.tile_pool(name="pqk", bufs=6) as pqk, \
             tc.tile_pool(name="pv", bufs=1) as pv, \
             tc.tile_pool(name="pst", bufs=2) as pst, \
             tc.tile_pool(name="ppt", bufs=16) as ppt, \
             tc.tile_pool(name="poT", bufs=2) as poT, \
             tc.tile_pool(name="pnm", bufs=2) as pnm, \
             tc.tile_pool(name="pout", bufs=3) as pout, \
             tc.tile_pool(name="psA", bufs=3, space="PSUM") as psA, \
             tc.tile_pool(name="ps5", bufs=2, space="PSUM") as ps5:

            # ---------------- phase 0: DMAs, constants, trig ------------------
            # All input DMA on the sync queue, interleaved so the first QKV
            # matmul (w0 + x0) can start ~2us in, with freqs early for trig.
            w_sb, wv_sb, xtk = [], [], []
            for kb in range(KB):
                wt = pw.tile([128, 4 * 128], BF16, tag="w", name=f"w{kb}")
                w_sb.append(wt)
                vt_ = pw.tile([128, 2 * 128], BF16, tag="wv", name=f"wv{kb}")
                wv_sb.append(vt_)
                t = pxT.tile([128, N], BF16, tag="xT", name=f"xT_{kb}")
                xtk.append(t)
            wo_sb = [pwo.tile([128, DIM], BF16, tag="wo", name=f"wo{i}")
                     for i in range(2)]
            rmat = pc.tile([128, 128], BF16, tag="rmat")
            xt = [[xtk[kb] for kb in range(KB)] for _ in range(2)]

            # persistent q/k/v/o tensors
            qT = [pqk.tile([128, N], BF16, tag="qk", name=f"qT{p}")
                  for p in range(2)]
            kTz = [pqk.tile([128, N], BF16, tag="qk", name=f"kTz{h}")
                   for h in range(HPC)]
            for h in range(2):
                hh = h % 2
                nc.gpsimd.memset(kTz[h][(1 - hh) * 64:(2 - hh) * 64, :], 0.0)

            vtall = pv.tile([128, HPC, NT, DH + 1], BF16, tag="v")
            nc.gpsimd.memset(vtall[:, :, :, DH:DH + 1], 1.0)
            oT = [poT.tile([128, N], BF16, tag="oT", name=f"oT{p}")
                  for p in range(2)]
            ones_f = pc.tile([1, 128], F32, tag="ones_f")
            nc.vector.memset(ones_f[:], 1.0)
            ones_r = pc.tile([1, 128], F32R, tag="ones_r")
            nc.vector.tensor_copy(ones_r[:], ones_f[:])

            # trig helper: sin/cos with range reduction, written to dst views
            def emit_trig(sin_dst, cos_dst, src, width):
                # round-to-nearest via the MAGIC trick; the tensor_scalar
                # steps run as ACT Copy (out = in*scale + bias), the
                # two-tensor step on the DVE.
                for which in range(2):  # 0: sin, 1: cos
                    if which == 1:
                        y = pout.tile([128, width], F32, tag="trig_x", bufs=2)
                        nc.scalar.activation(y[:], src, AF.Copy,
                                             bias=float(np.pi / 2))
                        yv = y[:]
                    else:
                        yv = src
                    k = pout.tile([128, width], F32, tag="trig_k", bufs=1)
                    nc.scalar.activation(k[:], yv, AF.Copy, bias=MAGIC,
                                         scale=float(1.0 / TWO_PI))
                    nc.scalar.activation(k[:], k[:], AF.Copy, bias=-MAGIC)
                    xr = pout.tile([128, width], F32, tag="trig_x", bufs=2)
                    nc.vector.scalar_tensor_tensor(xr[:], k[:], -TWO_PI, yv,
                                                   op0=OP.mult, op1=OP.add)
                    dst = cos_dst if which == 1 else sin_dst
                    nc.scalar.activation(dst, xr[:], AF.Sin)

            # cos/sin in [d, n] layout for q/k rotary.  fTF is freqs^T folded:
            # rows 0:64 = d for n in [0, 1024), rows 64:128 = n in [1024, 2048).
            # Compute trig on the folded layout, then unfold + duplicate the
            # head-halves with SBUF->SBUF DMAs.
            cosT2 = pc.tile([128, N], F32, tag="cosT2")
            sinT2 = pc.tile([128, N], F32, tag="sinT2")
            foldc = pc.tile([128, 1024], F32, tag="foldc")
            folds = pc.tile([128, 1024], F32, tag="folds")
            ftP = pc.tile([128, NT, DH], F32, tag="ftP")
            fview = fPD[:].rearrange("(t p) d -> p t d", p=128)
            # input DMA, ordered for earliest consumption: first w/x pair and
            # the freqs (trig is on the critical path to the first rotary),
            # then the remaining w/x blocks.
            nc.sync.dma_start(w_sb[0][:], wqkD[0:128, :])
            nc.sync.dma_start(xtk[0][:], xbT[0:128, :])
            nc.sync.dma_start(rmat[:], rmatD[:])
            nc.gpsimd.dma_start(foldc[:], fTFD[:])
            for kb in (1, 3, 5, 7):
                nc.gpsimd.dma_start(xtk[kb][:], xbT[kb * 128:(kb + 1) * 128, :])
            nc.gpsimd.dma_start(ftP[:, 0:NT // 2, :], fview[:, 0:NT // 2, :])
            nc.gpsimd.dma_start(ftP[:, NT // 2:NT, :], fview[:, NT // 2:NT, :])
            for kb in range(1, KB):
                nc.sync.dma_start(w_sb[kb][:], wqkD[kb * 128:(kb + 1) * 128, :])
                if kb >= 2 and kb % 2 == 0:
                    nc.sync.dma_start(xtk[kb][:],
                                      xbT[kb * 128:(kb + 1) * 128, :])
            for kb in range(KB):
                nc.sync.dma_start(wv_sb[kb][:], wvD[kb * 128:(kb + 1) * 128, :])
            for cb in range(2):
                nc.sync.dma_start(wo_sb[cb][:], woD[cb * 128:(cb + 1) * 128, :])
            emit_trig(folds[:], foldc[:], foldc[:], 1024)
            for t, f in ((cosT2, foldc), (sinT2, folds)):
                nc.sync.dma_start(t[0:64, 0:1024], f[0:64, :])
                nc.sync.dma_start(t[64:128, 0:1024], f[0:64, :])
                nc.sync.dma_start(t[0:64, 1024:2048], f[64:128, :])
                nc.sync.dma_start(t[64:128, 1024:2048], f[64:128, :])

            # cos/sin in [n, d] layout (even|odd permuted) for v rotary
            cosP = pc.tile([128, NT, DH], F32, tag="cosP")
            sinP = pc.tile([128, NT, DH], F32, tag="sinP")
            emit_trig(sinP[:].rearrange("p t d -> p (t d)"),
                      cosP[:].rearrange("p t d -> p (t d)"),
                      ftP[:].rearrange("p t d -> p (t d)"), NT * DH)
            warm = pnm.tile([1, 8], F32, tag="warm", bufs=1)
            nc.scalar.activation(warm[:], ones_f[0:1, 0:8], AF.Exp)

            # ---------------- QKV pieces (projection + rotary) ----------------
            # Stage 1: projection matmuls + bf16 casts (frees the psum fast).
            # Stage 2: rotary (R-matmul + combines) -- emitted later so the
            # casts have cleared the scalar queue and never stall the PE.
            def emit_qk_stage1(jt, cp):
                qps = psA.tile([128, 1024], F32, tag="psA")
                for kb in range(KB):
                    for mh in range(2):
                        nc.tensor.matmul(
                            qps[:, mh * 512:(mh + 1) * 512],
                            w_sb[kb][:, jt * 128:(jt + 1) * 128],
                            xtk[kb][:, cp * 1024 + mh * 512:
                                    cp * 1024 + (mh + 1) * 512],
                            start=(kb == 0), stop=(kb == KB - 1))
                ts = []
                for half in range(2):
                    t_sb = pst.tile([128, CW], BF16, tag="t_sb", bufs=4)
                    nc.scalar.copy(t_sb[:],
                                   qps[:, half * 512:(half + 1) * 512])
                    ts.append(t_sb)
                return ts

            def emit_qk_stage2(jt, cp, ts):
                pair = jt // 2
                is_q = (jt % 2) == 0
                for half in range(2):
                    c = cp * 2 + half
                    csl = cosT2[:, c * CW:(c + 1) * CW]
                    ssl = sinT2[:, c * CW:(c + 1) * CW]
                    t_sb = ts[half]
                    rps = psA.tile([128, CW], F32, tag="psA")
                    nc.tensor.matmul(rps[:], rmat[:], t_sb[:],
                                     start=True, stop=True)
                    tmp = pst.tile([128, CW], F32, tag="tmp", bufs=2)
                    nc.gpsimd.tensor_mul(tmp[:], t_sb[:], csl)
                    rs = pst.tile([128, CW], F32, tag="rs", bufs=2)
                    nc.vector.tensor_mul(rs[:], rps[:], ssl)
                    if is_q:
                        nc.gpsimd.tensor_add(
                            qT[pair][:, c * CW:(c + 1) * CW], tmp[:], rs[:])
                    else:
                        for hh in range(2):
                            h = pair * 2 + hh
                            sl = slice(hh * 64, (hh + 1) * 64)
                            nc.gpsimd.tensor_add(
                                kTz[h][sl, c * CW:(c + 1) * CW],
                                tmp[sl, :], rs[sl, :])

            def emit_qk_piece(jt, cp):
                emit_qk_stage2(jt, cp, emit_qk_stage1(jt, cp))

            def emit_v_block(tb):
                """v for both pairs, 2 row-tiles (nt = 2*tb, 2*tb+1)."""
                vps = ps5.tile([128, 2, 256], F32, tag="ps5", bufs=2)
                for i in range(2):
                    nt = 2 * tb + i
                    off = nt * 128
                    for kb in range(KB):
                        nc.tensor.matmul(
                            vps[:, i, :],
                            xtk[kb][:, off:off + 128],
                            wv_sb[kb][:],
                            start=(kb == 0), stop=(kb == KB - 1))
                if dbg and tb == 0:
                    vr = pout.tile([128, 512], F32, tag="dvraw", bufs=1)
                    nc.vector.tensor_copy(vr[:], vps[:].rearrange(
                        "p a b -> p (a b)"))
                    nc.sync.dma_start(dbgD["d_vraw"][:], vr[:])
                for i in range(2):
                    nt = 2 * tb + i
                    vv = vps[:, i, :].rearrange("p (h q s) -> p h q s",
                                                h=HPC, q=2)
                    ve, vo = vv[:, :, 0, :], vv[:, :, 1, :]
                    cE = cosP[:, nt, 0:32].unsqueeze(1).broadcast_to(
                        (128, HPC, 32))
                    cO = cosP[:, nt, 32:64].unsqueeze(1).broadcast_to(
                        (128, HPC, 32))
                    sE = sinP[:, nt, 0:32].unsqueeze(1).broadcast_to(
                        (128, HPC, 32))
                    sO = sinP[:, nt, 32:64].unsqueeze(1).broadcast_to(
                        (128, HPC, 32))
                    m1 = pst.tile([128, HPC, 32], F32, tag="vt1", bufs=2)
                    m2 = pst.tile([128, HPC, 32], F32, tag="vt2", bufs=2)
                    nc.vector.tensor_mul(m1[:], ve, cE)
                    nc.vector.tensor_mul(m2[:], vo, sE)
                    nc.vector.tensor_sub(vtall[:, :, nt, 0:32], m1[:], m2[:])
                    m3 = pst.tile([128, HPC, 32], F32, tag="vt1", bufs=2)
                    m4 = pst.tile([128, HPC, 32], F32, tag="vt2", bufs=2)
                    nc.vector.tensor_mul(m3[:], vo, cO)
                    nc.vector.tensor_mul(m4[:], ve, sO)
                    nc.vector.tensor_add(vtall[:, :, nt, 32:64], m3[:], m4[:])

            # ---------------- attention for one pair --------------------------
            def emit_attention(pair, fillers, order=None):
                """fillers: list of 4 callables, one slotted per chunk."""
                for ci, c in enumerate(order or range(NCH)):
                    nj = 4 * c + 4
                    pts = {}
                    for g in range(nj // 2):
                        j0 = g * 2
                        for hh in range(2):
                            h = pair * 2 + hh
                            sps = psA.tile([128, 1024], F32, tag="psA")
                            for gg in range(2):
                                j = j0 + gg
                                nc.tensor.matmul(
                                    sps[:, gg * 512:(gg + 1) * 512],
                                    kTz[h][:, j * 128:(j + 1) * 128],
                                    qT[pair][:, c * CW:(c + 1) * CW],
                                    start=True, stop=True)
                            pt = ppt.tile([128, 1024], BF16, tag="pt",
                                          bufs=16)
                            nc.scalar.activation(pt[:], sps[:], AF.Exp,
                                                 scale=SCALE)
                            if j0 + 1 >= 4 * c:  # group touches the diagonal
                                w0 = min(CW, (j0 + 2 - 4 * c) * 128)
                                ptv = pt[:].rearrange(
                                    "p (g i) -> p g i", g=2)[:, :, 0:w0]
                                nc.gpsimd.affine_select(
                                    out=ptv, in_=ptv,
                                    compare_op=OP.is_ge, fill=0.0,
                                    base=c * CW - j0 * 128,
                                    pattern=[[-128, 2], [1, w0]],
                                    channel_multiplier=-1)
                            pts[(g, hh)] = pt
                    if fillers[ci] is not None:
                        fillers[ci]()
                    avs = {}
                    for hh in range(2):
                        h = pair * 2 + hh
                        av = ps5.tile([DH + 1, CW], F32, tag="ps5", bufs=2)
                        for j in range(nj):
                            nc.tensor.matmul(
                                av[:], vtall[:, h, j, :],
                                pts[(j // 2, hh)][:, (j % 2) * 512:
                                                  (j % 2 + 1) * 512],
                                start=(j == 0), stop=(j == nj - 1))
                        avs[hh] = av
                    for hh in range(2):
                        av = avs[hh]
                        s_r = pnm.tile([1, CW], F32R, tag="s_r", bufs=2)
                        nc.vector.tensor_copy(s_r[:], av[DH:DH + 1, :])
                        rbp = psA.tile([64, CW], F32, tag="psA")
                        nc.tensor.matmul(rbp[:], ones_r[0:1, 0:64], s_r[:],
                                         start=True, stop=True)
                        rb = pnm.tile([64, CW], F32, tag="rb", bufs=2)
                        nc.vector.reciprocal_approx_fast(rb[:], rbp[:])
                        osl = oT[pair][hh * 64:(hh + 1) * 64,
                                       c * CW:(c + 1) * CW]
                        nc.vector.tensor_mul(osl, av[0:DH, :], rb[:])

            # ---------------- output projection -------------------------------
            def emit_proj(c):
                for nt in range(4 * c, 4 * c + 4):
                    prj = psA.tile([128, DIM], F32, tag="psA")
                    for mh in range(2):
                        for cb in range(2):
                            nc.tensor.matmul(
                                prj[:, mh * 512:(mh + 1) * 512],
                                oT[cb][:, nt * 128:(nt + 1) * 128],
                                wo_sb[cb][:, mh * 512:(mh + 1) * 512],
                                start=(cb == 0), stop=(cb == 1))
                    ot = pout.tile([128, DIM], BF16, tag="osb", bufs=2)
                    if nt % 2 == 0:
                        nc.scalar.copy(ot[:], prj[:])
                    else:
                        nc.vector.tensor_copy(ot[:], prj[:])
                    nc.sync.dma_start(outD[nt * 128:(nt + 1) * 128, :], ot[:])

            # ---------------- schedule ----------------------------------------
            # phase 1a: pair-0 q/k, then v for both pairs
            for jt in (0, 1):
                for cp in range(2):
                    emit_qk_piece(jt, cp)
            for tb in range(NT // 2):
                emit_v_block(tb)

            # phase 2a: pair-0 attention with pair-1 qkv as PE filler.
            # Rotary of piece k runs one chunk after its projection so the
            # bf16 casts never gate the PE stream.
            pieces = [(2, 0), (2, 1), (3, 0), (3, 1)]
            pending_qk = {}

            def mk_fill(ci):
                def f():
                    if ci == 1:
                        for h in (2, 3):
                            nc.gpsimd.memset(
                                kTz[h][(1 - h % 2) * 64:(2 - h % 2) * 64, :],
                                0.0)
                    if ci > 0:
                        emit_qk_stage2(*pieces[ci - 1], pending_qk.pop(ci - 1))
                    pending_qk[ci] = emit_qk_stage1(*pieces[ci])
                return f
            emit_attention(0, [mk_fill(0), mk_fill(1), mk_fill(2), mk_fill(3)])

            # phase 2b: pair-1 attention, biggest chunk first so the tail is
            # the small chunk; output projection slots in as PE filler.
            prj_fillers = [
                (lambda: emit_qk_stage2(*pieces[3], pending_qk.pop(3))),
                (lambda: emit_proj(3)),
                (lambda: emit_proj(2)),
                (lambda: emit_proj(1))]
            emit_attention(1, prj_fillers, order=[3, 2, 1, 0])
            emit_proj(0)

            if dbg:
                nc.sync.dma_start(dbgD["d_cosT2"][:], cosT2[:])
                nc.sync.dma_start(dbgD["d_sinT2"][:], sinT2[:])
                nc.sync.dma_start(dbgD["d_cosP"][:],
                                  cosP[:].rearrange("p t d -> p (t d)"))
                nc.sync.dma_start(dbgD["d_qT0"][:], qT[0][:])
                nc.sync.dma_start(dbgD["d_kTz0"][:], kTz[0][:])
                nc.sync.dma_start(dbgD["d_kTz1"][:], kTz[1][:])
                nc.sync.dma_start(dbgD["d_vtall"][:],
                                  vtall[:].rearrange("p h t d -> p (h t d)"))
                nc.sync.dma_start(dbgD["d_oT0"][:], oT[0][:])

    nc.compile()
    return nc


def _get_program():
    if "nc" not in _CACHE:
        _CACHE["nc"] = _build_program()
    return _CACHE["nc"]


def _rot_lhsT():
    """lhsT for rot_half: out = lhsT.T @ tT = R @ tT, interleaved pairs."""
    R64 = np.zeros((64, 64), np.float32)
    for i in range(32):
        R64[2 * i, 2 * i + 1] = -1.0
        R64[2 * i + 1, 2 * i] = 1.0
    R = np.zeros((128, 128), np.float32)
    R[0:64, 0:64] = R64
    R[64:128, 64:128] = R64
    return np.ascontiguousarray(R.T)


PERM_EO = list(range(0, DH, 2)) + list(range(1, DH, 2))


def make_in_maps(x, rotary_pos_emb, w_qkv, w_out, b_out):
    x = np.asarray(x, np.float32)
    rotary_pos_emb = np.asarray(rotary_pos_emb, np.float32)
    w_qkv = np.asarray(w_qkv, np.float32)
    w_out = np.asarray(w_out, np.float32)

    import ml_dtypes
    bf16 = ml_dtypes.bfloat16
    rmatT = _rot_lhsT().astype(bf16)
    fT = np.ascontiguousarray(rotary_pos_emb.T)               # [64, 2048]
    fTF = np.ascontiguousarray(
        np.concatenate([fT[:, :N // 2], fT[:, N // 2:]], axis=0))
    fP = np.ascontiguousarray(rotary_pos_emb[:, PERM_EO])

    in_maps = []
    for c in range(NCORES):
        b = c // 4
        heads = [4 * (c % 4) + i for i in range(HPC)]
        qk_cols = []
        for p in range(2):
            for t in range(2):          # q, k
                for hh in range(2):
                    h = heads[2 * p + hh]
                    qk_cols.append(
                        w_qkv[:, t * H * DH + h * DH:t * H * DH + (h + 1) * DH])
        wqk = np.ascontiguousarray(np.concatenate(qk_cols, axis=1))
        v_cols = []
        for p in range(2):
            for hh in range(2):
                h = heads[2 * p + hh]
                blk = w_qkv[:, 2 * H * DH + h * DH:2 * H * DH + (h + 1) * DH]
                v_cols.append(blk[:, PERM_EO])
        wv = np.ascontiguousarray(np.concatenate(v_cols, axis=1))
        wo_rows = []
        for p in range(2):
            for hh in range(2):
                h = heads[2 * p + hh]
                wo_rows.append(w_out[h * DH:(h + 1) * DH, :][PERM_EO, :])
        wo = np.ascontiguousarray(np.concatenate(wo_rows, axis=0))
        in_maps.append({
            "xbT": np.ascontiguousarray(x[b].T).astype(bf16),
            "wqk": wqk.astype(bf16),
            "wv": wv.astype(bf16),
            "wo": wo.astype(bf16),
            "fTF": fTF,
            "fP": fP,
            "rmatD": rmatT,
        })
    return in_maps


def gather(res, b_out):
    out = np.zeros((B, N, DIM), np.float32)
    for c in range(NCORES):
        out[c // 4] += np.asarray(res[c]["out"], np.float32)
    out += np.asarray(b_out, np.float32)[None, None, :]
    return out


def kernel(x, rotary_pos_emb, w_qkv, w_out, b_out):
    from concourse.bass_utils import run_bass_kernel_spmd

    nc = _get_program()
    in_maps = make_in_maps(x, rotary_pos_emb, w_qkv, w_out, b_out)
    res = run_bass_kernel_spmd(nc, in_maps, list(range(NCORES))).results
    return gather(res, b_out)


# revision 42
# speedup vs baseline: 1.0390x; 1.0390x over previous
"""Trainium2 Bass kernel for nn_Attention_43946105373274.

Causal multi-head attention with rotary embeddings applied to q, k and v.
B=2, N=2048, DIM=1024, H=16, DH=64, f32.

Sharding: 8 cores = (2 batches) x (4 head-groups of 4 heads).
Each core computes the qkv projection for its heads (w_qkv column-shard),
full causal attention for its heads, and a partial output projection
(w_out row-shard).  The host sums the 4 partials per batch and adds the
bias -- full inputs in, full output out.

v2 design notes (vs the 240us baseline):
  - All matmuls run in the 128x128 PE tile mode: per-head k tiles are
    zero-padded to 128 contraction rows (kTz) so the S^T matmuls no
    longer flip the array into 64x128 mode (mode switches drain the PE).
  - No PE transposes at all: cos/sin for the [d, n] layout are computed
    from a host-folded freqs^T input and duplicated across head-halves
    with SBUF->SBUF DMAs; V is computed directly in [n, d] layout with
    w_v as the moving operand and an even/odd-permuted column order so
    rotate_half becomes contiguous half-tile DVE ops.
  - Bias matmuls removed (host adds b_out during the partial-sum gather).
  - Software pipelining: pair-1 QKV runs as PE filler inside pair-0's
    scalar-bound attention; the output projection overlaps pair-1's
    attention.  x^T DMA transposes split across the sync+scalar HWDGE
    queues.
  - Output written as bf16 (partials summed in f32 on the host).
"""

import sys
import numpy as np

if "/opt/trn_rl_repo" not in sys.path:
    sys.path.insert(0, "/opt/trn_rl_repo")

B, N, DIM, H, DH = 2, 2048, 1024, 16, 64
HPC = 4                     # heads per core
NCORES = 8
SCALE = DH ** -0.5
NT = N // 128               # 16 row tiles
KB = DIM // 128             # 8 contraction blocks
CW = 512                    # i-chunk width
NCH = N // CW               # 4 chunks

_CACHE = {}


def _build_program():
    import concourse.bass as bass  # noqa: F401
    import concourse.mybir as mybir
    import concourse.tile as tile
    from concourse import bacc

    F32 = mybir.dt.float32
    F32R = mybir.dt.float32r
    BF16 = mybir.dt.bfloat16
    AF = mybir.ActivationFunctionType
    OP = mybir.AluOpType

    nc = bacc.Bacc("TRN2", target_bir_lowering=False, debug=False,
                   num_devices=NCORES)

    xbT = nc.dram_tensor("xbT", [DIM, N], BF16, kind="ExternalInput")
    wqkD = nc.dram_tensor("wqk", [DIM, 4 * 128], BF16, kind="ExternalInput")
    wvD = nc.dram_tensor("wv", [DIM, 2 * 128], BF16, kind="ExternalInput")
    woD = nc.dram_tensor("wo", [HPC * DH, DIM], BF16, kind="ExternalInput")
    fTFD = nc.dram_tensor("fTF", [128, N // 2], F32, kind="ExternalInput")
    fPD = nc.dram_tensor("fP", [N, DH], F32, kind="ExternalInput")
    rmatD = nc.dram_tensor("rmatD", [128, 128], BF16, kind="ExternalInput")
    outD = nc.dram_tensor("out", [N, DIM], BF16, kind="ExternalOutput")
    import os
    dbg = os.environ.get("KDEBUG", "0") == "1"
    if dbg:
        dbgD = {
            "d_cosT2": nc.dram_tensor("d_cosT2", [128, N], F32, kind="ExternalOutput"),
            "d_sinT2": nc.dram_tensor("d_sinT2", [128, N], F32, kind="ExternalOutput"),
            "d_cosP": nc.dram_tensor("d_cosP", [128, NT * DH], F32, kind="ExternalOutput"),
            "d_qT0": nc.dram_tensor("d_qT0", [128, N], BF16, kind="ExternalOutput"),
            "d_kTz0": nc.dram_tensor("d_kTz0", [128, N], BF16, kind="ExternalOutput"),
            "d_kTz1": nc.dram_tensor("d_kTz1", [128, N], BF16, kind="ExternalOutput"),
            "d_vtall": nc.dram_tensor("d_vtall", [128, HPC * NT * (DH + 1)], BF16, kind="ExternalOutput"),
            "d_oT0": nc.dram_tensor("d_oT0", [128, N], BF16, kind="ExternalOutput"),
            "d_vraw": nc.dram_tensor("d_vraw", [128, 512], F32, kind="ExternalOutput"),
        }

    MAGIC = 12582912.0          # 1.5 * 2**23: float32 round-to-nearest trick
    TWO_PI = float(2 * np.pi)

    with tile.TileContext(nc) as tc:
        with tc.tile_pool(name="pc", bufs=1) as pc, \
             tc.tile_pool(name="pw", bufs=16) as pw, \
             tc.tile_pool(name="pwo", bufs=2) as pwo, \
             tc.tile_pool(name="pxT", bufs=8) as pxT, \
             tc.tile_pool(name="pqk", bufs=6) as pqk, \
             tc.tile_pool(name="pv", bufs=1) as pv, \
             tc.tile_pool(name="pst", bufs=2) as pst, \
             tc.tile_pool(name="ppt", bufs=16) as ppt, \
             tc.tile_pool(name="poT", bufs=2) as poT, \
             tc.tile_pool(name="pnm", bufs=2) as pnm, \
             tc.tile_pool(name="pout", bufs=3) as pout, \
             tc.tile_pool(name="psA", bufs=3, space="PSUM") as psA, \
             tc.tile_pool(name="ps5", bufs=2, space="PSUM") as ps5:

            # ---------------- phase 0: DMAs, constants, trig ------------------
            # All input DMA on the sync queue, interleaved so the first QKV
            # matmul (w0 + x0) can start ~2us in, with freqs early for trig.
            w_sb, wv_sb, xtk = [], [], []
            for kb in range(KB):
                wt = pw.tile([128, 4 * 128], BF16, tag="w", name=f"w{kb}")
                w_sb.append(wt)
                vt_ = pw.tile([128, 2 * 128], BF16, tag="wv", name=f"wv{kb}")
                wv_sb.append(vt_)
                t = pxT.tile([128, N], BF16, tag="xT", name=f"xT_{kb}")
                xtk.append(t)
            wo_sb = [pwo.tile([128, DIM], BF16, tag="wo", name=f"wo{i}")
                     for i in range(2)]
            rmat = pc.tile([128, 128], BF16, tag="rmat")
            xt = [[xtk[kb] for kb in range(KB)] for _ in range(2)]

            # persistent q/k/v/o tensors
            qT = [pqk.tile([128, N], BF16, tag="qk", name=f"qT{p}")
                  for p in range(2)]
            kTz = [pqk.tile([128, N], BF16, tag="qk", name=f"kTz{h}")
                   for h in range(HPC)]
            for h in range(2):
                hh = h % 2
                nc.gpsimd.memset(kTz[h][(1 - hh) * 64:(2 - hh) * 64, :], 0.0)

            vtall = pv.tile([128, HPC, NT, DH + 1], BF16, tag="v")
            nc.gpsimd.memset(vtall[:, :, :, DH:DH + 1], 1.0)
            oT = [poT.tile([128, N], BF16, tag="oT", name=f"oT{p}")
                  for p in range(2)]
            ones_f = pc.tile([1, 128], F32, tag="ones_f")
            nc.vector.memset(ones_f[:], 1.0)
            ones_r = pc.tile([1, 128], F32R, tag="ones_r")
            nc.vector.tensor_copy(ones_r[:], ones_f[:])

            # trig helper: sin/cos with range reduction, written to dst views
            def emit_trig(sin_dst, cos_dst, src, width):
                # round-to-nearest via the MAGIC trick; the tensor_scalar
                # steps run as ACT Copy (out = in*scale + bias), the
                # two-tensor step on the DVE.
                for which in range(2):  # 0: sin, 1: cos
                    if which == 1:
                        y = pout.tile([128, width], F32, tag="trig_x", bufs=2)
                        nc.scalar.activation(y[:], src, AF.Copy,
                                             bias=float(np.pi / 2))
                        yv = y[:]
                    else:
                        yv = src
                    k = pout.tile([128, width], F32, tag="trig_k", bufs=1)
                    nc.scalar.activation(k[:], yv, AF.Copy, bias=MAGIC,
                                         scale=float(1.0 / TWO_PI))
                    nc.scalar.activation(k[:], k[:], AF.Copy, bias=-MAGIC)
                    xr = pout.tile([128, width], F32, tag="trig_x", bufs=2)
                    nc.vector.scalar_tensor_tensor(xr[:], k[:], -TWO_PI, yv,
                                                   op0=OP.mult, op1=OP.add)
                    dst = cos_dst if which == 1 else sin_dst
                    nc.scalar.activation(dst, xr[:], AF.Sin)

            # cos/sin in [d, n] layout for q/k rotary.  fTF is freqs^T folded:
            # rows 0:64 = d for n in [0, 1024), rows 64:128 = n in [1024, 2048).
            # Compute trig on the folded layout, then unfold + duplicate the
            # head-halves with SBUF->SBUF DMAs.
            cosT2 = pc.tile([128, N], F32, tag="cosT2")
            sinT2 = pc.tile([128, N], F32, tag="sinT2")
            foldc = pc.tile([128, 1024], F32, tag="foldc")
            folds = pc.tile([128, 1024], F32, tag="folds")
            ftP = pc.tile([128, NT, DH], F32, tag="ftP")
            fview = fPD[:].rearrange("(t p) d -> p t d", p=128)
            # input DMA, ordered for earliest consumption: first w/x pair and
            # the freqs (trig is on the critical path to the first rotary),
            # then the remaining w/x blocks.
            nc.sync.dma_start(w_sb[0][:], wqkD[0:128, :])
            nc.sync.dma_start(xtk[0][:], xbT[0:128, :])
            nc.sync.dma_start(rmat[:], rmatD[:])
            nc.sync.dma_start(xtk[1][:], xbT[128:256, :])
            nc.gpsimd.dma_start(foldc[:], fTFD[:])
            nc.gpsimd.dma_start(ftP[:, 0:NT // 2, :], fview[:, 0:NT // 2, :])
            nc.gpsimd.dma_start(ftP[:, NT // 2:NT, :], fview[:, NT // 2:NT, :])
            for kb in range(1, KB):
                nc.sync.dma_start(w_sb[kb][:], wqkD[kb * 128:(kb + 1) * 128, :])
                if kb >= 2:
                    nc.sync.dma_start(xtk[kb][:],
                                      xbT[kb * 128:(kb + 1) * 128, :])
            for kb in range(KB):
                nc.sync.dma_start(wv_sb[kb][:], wvD[kb * 128:(kb + 1) * 128, :])
            for cb in range(2):
                nc.sync.dma_start(wo_sb[cb][:], woD[cb * 128:(cb + 1) * 128, :])
            emit_trig(folds[:], foldc[:], foldc[:], 1024)
            for t, f in ((cosT2, foldc), (sinT2, folds)):
                nc.sync.dma_start(t[0:64, 0:1024], f[0:64, :])
                nc.sync.dma_start(t[64:128, 0:1024], f[0:64, :])
                nc.sync.dma_start(t[0:64, 1024:2048], f[64:128, :])
                nc.sync.dma_start(t[64:128, 1024:2048], f[64:128, :])

            # cos/sin in [n, d] layout (even|odd permuted) for v rotary
            cosP = pc.tile([128, NT, DH], F32, tag="cosP")
            sinP = pc.tile([128, NT, DH], F32, tag="sinP")
            emit_trig(sinP[:].rearrange("p t d -> p (t d)"),
                      cosP[:].rearrange("p t d -> p (t d)"),
                      ftP[:].rearrange("p t d -> p (t d)"), NT * DH)
            warm = pnm.tile([1, 8], F32, tag="warm", bufs=1)
            nc.scalar.activation(warm[:], ones_f[0:1, 0:8], AF.Exp)

            # ---------------- QKV pieces (projection + rotary) ----------------
            # Stage 1: projection matmuls + bf16 casts (frees the psum fast).
            # Stage 2: rotary (R-matmul + combines) -- emitted later so the
            # casts have cleared the scalar queue and never stall the PE.
            def emit_qk_stage1(jt, cp):
                qps = psA.tile([128, 1024], F32, tag="psA")
                for kb in range(KB):
                    for mh in range(2):
                        nc.tensor.matmul(
                            qps[:, mh * 512:(mh + 1) * 512],
                            w_sb[kb][:, jt * 128:(jt + 1) * 128],
                            xtk[kb][:, cp * 1024 + mh * 512:
                                    cp * 1024 + (mh + 1) * 512],
                            start=(kb == 0), stop=(kb == KB - 1))
                ts = []
                for half in range(2):
                    t_sb = pst.tile([128, CW], BF16, tag="t_sb", bufs=4)
                    nc.scalar.copy(t_sb[:],
                                   qps[:, half * 512:(half + 1) * 512])
                    ts.append(t_sb)
                return ts

            def emit_qk_stage2(jt, cp, ts):
                pair = jt // 2
                is_q = (jt % 2) == 0
                for half in range(2):
                    c = cp * 2 + half
                    csl = cosT2[:, c * CW:(c + 1) * CW]
                    ssl = sinT2[:, c * CW:(c + 1) * CW]
                    t_sb = ts[half]
                    rps = psA.tile([128, CW], F32, tag="psA")
                    nc.tensor.matmul(rps[:], rmat[:], t_sb[:],
                                     start=True, stop=True)
                    tmp = pst.tile([128, CW], F32, tag="tmp", bufs=2)
                    nc.gpsimd.tensor_mul(tmp[:], t_sb[:], csl)
                    rs = pst.tile([128, CW], F32, tag="rs", bufs=2)
                    nc.vector.tensor_mul(rs[:], rps[:], ssl)
                    if is_q:
                        nc.gpsimd.tensor_add(
                            qT[pair][:, c * CW:(c + 1) * CW], tmp[:], rs[:])
                    else:
                        for hh in range(2):
                            h = pair * 2 + hh
                            sl = slice(hh * 64, (hh + 1) * 64)
                            nc.gpsimd.tensor_add(
                                kTz[h][sl, c * CW:(c + 1) * CW],
                                tmp[sl, :], rs[sl, :])

            def emit_qk_piece(jt, cp):
                emit_qk_stage2(jt, cp, emit_qk_stage1(jt, cp))

            def emit_v_block(tb):
                """v for both pairs, 2 row-tiles (nt = 2*tb, 2*tb+1)."""
                vps = ps5.tile([128, 2, 256], F32, tag="ps5", bufs=2)
                for i in range(2):
                    nt = 2 * tb + i
                    off = nt * 128
                    for kb in range(KB):
                        nc.tensor.matmul(
                            vps[:, i, :],
                            xtk[kb][:, off:off + 128],
                            wv_sb[kb][:],
                            start=(kb == 0), stop=(kb == KB - 1))
                if dbg and tb == 0:
                    vr = pout.tile([128, 512], F32, tag="dvraw", bufs=1)
                    nc.vector.tensor_copy(vr[:], vps[:].rearrange(
                        "p a b -> p (a b)"))
                    nc.sync.dma_start(dbgD["d_vraw"][:], vr[:])
                for i in range(2):
                    nt = 2 * tb + i
                    vv = vps[:, i, :].rearrange("p (h q s) -> p h q s",
                                                h=HPC, q=2)
                    ve, vo = vv[:, :, 0, :], vv[:, :, 1, :]
                    cE = cosP[:, nt, 0:32].unsqueeze(1).broadcast_to(
                        (128, HPC, 32))
                    cO = cosP[:, nt, 32:64].unsqueeze(1).broadcast_to(
                        (128, HPC, 32))
                    sE = sinP[:, nt, 0:32].unsqueeze(1).broadcast_to(
                        (128, HPC, 32))
                    sO = sinP[:, nt, 32:64].unsqueeze(1).broadcast_to(
                        (128, HPC, 32))
                    m1 = pst.tile([128, HPC, 32], F32, tag="vt1", bufs=2)
                    m2 = pst.tile([128, HPC, 32], F32, tag="vt2", bufs=2)
                    nc.vector.tensor_mul(m1[:], ve, cE)
                    nc.vector.tensor_mul(m2[:], vo, sE)
                    nc.vector.tensor_sub(vtall[:, :, nt, 0:32], m1[:], m2[:])
                    m3 = pst.tile([128, HPC, 32], F32, tag="vt1", bufs=2)
                    m4 = pst.tile([128, HPC, 32], F32, tag="vt2", bufs=2)
                    nc.vector.tensor_mul(m3[:], vo, cO)
                    nc.vector.tensor_mul(m4[:], ve, sO)
                    nc.vector.tensor_add(vtall[:, :, nt, 32:64], m3[:], m4[:])

            # ---------------- attention for one pair --------------------------
            def emit_attention(pair, fillers, order=None):
                """fillers: list of 4 callables, one slotted per chunk."""
                for ci, c in enumerate(order or range(NCH)):
                    nj = 4 * c + 4
                    pts = {}
                    for g in range(nj // 2):
                        j0 = g * 2
                        for hh in range(2):
                            h = pair * 2 + hh
                            sps = psA.tile([128, 1024], F32, tag="psA")
                            for gg in range(2):
                                j = j0 + gg
                                nc.tensor.matmul(
                                    sps[:, gg * 512:(gg + 1) * 512],
                                    kTz[h][:, j * 128:(j + 1) * 128],
                                    qT[pair][:, c * CW:(c + 1) * CW],
                                    start=True, stop=True)
                            pt = ppt.tile([128, 1024], BF16, tag="pt",
                                          bufs=16)
                            nc.scalar.activation(pt[:], sps[:], AF.Exp,
                                                 scale=SCALE)
                            if j0 + 1 >= 4 * c:  # group touches the diagonal
                                w0 = min(CW, (j0 + 2 - 4 * c) * 128)
                                ptv = pt[:].rearrange(
                                    "p (g i) -> p g i", g=2)[:, :, 0:w0]
                                nc.gpsimd.affine_select(
                                    out=ptv, in_=ptv,
                                    compare_op=OP.is_ge, fill=0.0,
                                    base=c * CW - j0 * 128,
                                    pattern=[[-128, 2], [1, w0]],
                                    channel_multiplier=-1)
                            pts[(g, hh)] = pt
                    if fillers[ci] is not None:
                        fillers[ci]()
                    avs = {}
                    for hh in range(2):
                        h = pair * 2 + hh
                        av = ps5.tile([DH + 1, CW], F32, tag="ps5", bufs=2)
                        for j in range(nj):
                            nc.tensor.matmul(
                                av[:], vtall[:, h, j, :],
                                pts[(j // 2, hh)][:, (j % 2) * 512:
                                                  (j % 2 + 1) * 512],
                                start=(j == 0), stop=(j == nj - 1))
                        avs[hh] = av
                    for hh in range(2):
                        av = avs[hh]
                        s_r = pnm.tile([1, CW], F32R, tag="s_r", bufs=2)
                        nc.vector.tensor_copy(s_r[:], av[DH:DH + 1, :])
                        rbp = psA.tile([64, CW], F32, tag="psA")
                        nc.tensor.matmul(rbp[:], ones_r[0:1, 0:64], s_r[:],
                                         start=True, stop=True)
                        rb = pnm.tile([64, CW], F32, tag="rb", bufs=2)
                        nc.vector.reciprocal_approx_fast(rb[:], rbp[:])
                        osl = oT[pair][hh * 64:(hh + 1) * 64,
                                       c * CW:(c + 1) * CW]
                        nc.vector.tensor_mul(osl, av[0:DH, :], rb[:])

            # ---------------- output projection -------------------------------
            def emit_proj(c):
                for nt in range(4 * c, 4 * c + 4):
                    prj = psA.tile([128, DIM], F32, tag="psA")
                    for mh in range(2):
                        for cb in range(2):
                            nc.tensor.matmul(
                                prj[:, mh * 512:(mh + 1) * 512],
                                oT[cb][:, nt * 128:(nt + 1) * 128],
                                wo_sb[cb][:, mh * 512:(mh + 1) * 512],
                                start=(cb == 0), stop=(cb == 1))
                    ot = pout.tile([128, DIM], BF16, tag="osb", bufs=2)
                    if nt % 2 == 0:
                        nc.scalar.copy(ot[:], prj[:])
                    else:
                        nc.vector.tensor_copy(ot[:], prj[:])
                    nc.sync.dma_start(outD[nt * 128:(nt + 1) * 128, :], ot[:])

            # ---------------- schedule ----------------------------------------
            # phase 1a: pair-0 q/k, then v for both pairs
            for jt in (0, 1):
                for cp in range(2):
                    emit_qk_piece(jt, cp)
            for tb in range(NT // 2):
                emit_v_block(tb)

            # phase 2a: pair-0 attention with pair-1 qkv as PE filler.
            # Rotary of piece k runs one chunk after its projection so the
            # bf16 casts never gate the PE stream.
            pieces = [(2, 0), (2, 1), (3, 0), (3, 1)]
            pending_qk = {}

            def mk_fill(ci):
                def f():
                    if ci == 1:
                        for h in (2, 3):
                            nc.gpsimd.memset(
                                kTz[h][(1 - h % 2) * 64:(2 - h % 2) * 64, :],
                                0.0)
                    if ci > 0:
                        emit_qk_stage2(*pieces[ci - 1], pending_qk.pop(ci - 1))
                    pending_qk[ci] = emit_qk_stage1(*pieces[ci])
                return f
            emit_attention(0, [mk_fill(0), mk_fill(1), mk_fill(2), mk_fill(3)])

            # phase 2b: pair-1 attention, biggest chunk first so the tail is
            # the small chunk; output projection slots in as PE filler.
            prj_fillers = [
                (lambda: emit_qk_stage2(*pieces[3], pending_qk.pop(3))),
                (lambda: emit_proj(3)),
                (lambda: emit_proj(2)),
                (lambda: emit_proj(1))]
            emit_attention(1, prj_fillers, order=[3, 2, 1, 0])
            emit_proj(0)

            if dbg:
                nc.sync.dma_start(dbgD["d_cosT2"][:], cosT2[:])
                nc.sync.dma_start(dbgD["d_sinT2"][:], sinT2[:])
                nc.sync.dma_start(dbgD["d_cosP"][:],
                                  cosP[:].rearrange("p t d -> p (t d)"))
                nc.sync.dma_start(dbgD["d_qT0"][:], qT[0][:])
                nc.sync.dma_start(dbgD["d_kTz0"][:], kTz[0][:])
                nc.sync.dma_start(dbgD["d_kTz1"][:], kTz[1][:])
                nc.sync.dma_start(dbgD["d_vtall"][:],
                                  vtall[:].rearrange("p h t d -> p (h t d)"))
                nc.sync.dma_start(dbgD["d_oT0"][:], oT[0][:])

    nc.compile()
    return nc


def _get_program():
    if "nc" not in _CACHE:
        _CACHE["nc"] = _build_program()
    return _CACHE["nc"]


def _rot_lhsT():
    """lhsT for rot_half: out = lhsT.T @ tT = R @ tT, interleaved pairs."""
    R64 = np.zeros((64, 64), np.float32)
    for i in range(32):
        R64[2 * i, 2 * i + 1] = -1.0
        R64[2 * i + 1, 2 * i] = 1.0
    R = np.zeros((128, 128), np.float32)
    R[0:64, 0:64] = R64
    R[64:128, 64:128] = R64
    return np.ascontiguousarray(R.T)


PERM_EO = list(range(0, DH, 2)) + list(range(1, DH, 2))


def make_in_maps(x, rotary_pos_emb, w_qkv, w_out, b_out):
    x = np.asarray(x, np.float32)
    rotary_pos_emb = np.asarray(rotary_pos_emb, np.float32)
    w_qkv = np.asarray(w_qkv, np.float32)
    w_out = np.asarray(w_out, np.float32)

    import ml_dtypes
    bf16 = ml_dtypes.bfloat16
    rmatT = _rot_lhsT().astype(bf16)
    fT = np.ascontiguousarray(rotary_pos_emb.T)               # [64, 2048]
    fTF = np.ascontiguousarray(
        np.concatenate([fT[:, :N // 2], fT[:, N // 2:]], axis=0))
    fP = np.ascontiguousarray(rotary_pos_emb[:, PERM_EO])

    in_maps = []
    for c in range(NCORES):
        b = c // 4
        heads = [4 * (c % 4) + i for i in range(HPC)]
        qk_cols = []
        for p in range(2):
            for t in range(2):          # q, k
                for hh in range(2):
                    h = heads[2 * p + hh]
                    qk_cols.append(
                        w_qkv[:, t * H * DH + h * DH:t * H * DH + (h + 1) * DH])
        wqk = np.ascontiguousarray(np.concatenate(qk_cols, axis=1))
        v_cols = []
        for p in range(2):
            for hh in range(2):
                h = heads[2 * p + hh]
                blk = w_qkv[:, 2 * H * DH + h * DH:2 * H * DH + (h + 1) * DH]
                v_cols.append(blk[:, PERM_EO])
        wv = np.ascontiguousarray(np.concatenate(v_cols, axis=1))
        wo_rows = []
        for p in range(2):
            for hh in range(2):
                h = heads[2 * p + hh]
                wo_rows.append(w_out[h * DH:(h + 1) * DH, :][PERM_EO, :])
        wo = np.ascontiguousarray(np.concatenate(wo_rows, axis=0))
        in_maps.append({
            "xbT": np.ascontiguousarray(x[b].T).astype(bf16),
            "wqk": wqk.astype(bf16),
            "wv": wv.astype(bf16),
            "wo": wo.astype(bf16),
            "fTF": fTF,
            "fP": fP,
            "rmatD": rmatT,
        })
    return in_maps


def gather(res, b_out):
    out = np.zeros((B, N, DIM), np.float32)
    for c in range(NCORES):
        out[c // 4] += np.asarray(res[c]["out"], np.float32)
    out += np.asarray(b_out, np.float32)[None, None, :]
    return out


def kernel(x, rotary_pos_emb, w_qkv, w_out, b_out):
    from concourse.bass_utils import run_bass_kernel_spmd

    nc = _get_program()
    in_maps = make_in_maps(x, rotary_pos_emb, w_qkv, w_out, b_out)
    res = run_bass_kernel_spmd(nc, in_maps, list(range(NCORES))).results
    return gather(res, b_out)
